# revision 41
# baseline (speedup 1.0000x reference)
"""Causal attention (dense transformer block) on 8 Trainium2 NeuronCores.

Problem: x (4, 256, 64, 64) fp32; 1x1-conv q/kv projections; 8-head causal
attention over S = 64*64 = 4096 flattened pixels (head_dim 32); output
projection.  Full inputs in, full output out.

Sharding: 8 cores = 4 batches x 2 head-groups (4 heads each).  Each core
computes q/k/v projections for its head group, causal attention with scores
kept transposed (k-positions on partitions, q-positions on free dim, so
softmax denominators come out of the AV matmul via an appended ones column),
and a partial output projection.  Host sums the two head-group partials per
batch and adds the output bias.

Design (HW-microbenchmarked on the target cores):
  - QK matmuls (contraction = head_dim 32) run as 2-head concurrent slots
    via PE row tiling: operands live at partitions 32h so tile_position row
    groups 0/32/64 auto-derive; head 3 passes (96, 0) explicitly.  bf16
    operands keep LDWEIGHTS a separate, overlappable (FWL) instruction.
  - Scores stream through a bufs=2 pool of 2-bank psum tiles; one exp call
    per slot ([128, 2, width] strided AP, causally-valid width only) so
    ScalarE runs long back-to-back activations.
  - The causal mask is one shared 128x128 triangle applied post-exp on
    gpsimd (Pool engine, otherwise idle) to the leading 128 columns of
    diagonal slots only (128-granular diagonal).
  - AV matmuls accumulate per head into 4 plain [33, 512] psum banks
    (32 v-dims + a ones column that yields the softmax denominators),
    trailing the exp stream by AV_LAG k-tiles so row turnover hides the
    normalization latency.  Col-tiled packing measured slower; avoided.
  - Normalization reads PSUM directly: denominator -> partition 0, DVE
    reciprocal, gpsimd partition-broadcast, one DVE multiply per head.
  - k-bias is dropped entirely (it cancels in softmax); q-bias rides a DVE
    tensor_scalar add.  Projections for row j+1 and norms for row j-1 are
    emitted behind row j's first exp slots to keep ScalarE fed.
"""

import math
from contextlib import ExitStack

import numpy as np

import concourse.bass as bass
import concourse.tile as tile
from concourse import bacc, mybir

N_CORES = 8
N, C, HH, WW = 4, 256, 64, 64
S = HH * WW            # 4096
E = 256                # q/k width
O = 256                # v/out width
H = 8                  # heads
HD = E // H            # 32 head dim
HG = 4                 # heads per core
P = 128                # partitions
QC = 512               # q-chunk (psum bank width in fp32)
KT = 128               # k-tile
NQ = S // QC           # 8 q-chunks
NB = 6                 # psum banks in the QK/proj ring

F32 = mybir.dt.float32
F32R = mybir.dt.float32r
BF16 = mybir.dt.bfloat16


def build_kernel(reps=1, dbg=False, ablate=()):
    nc = bacc.Bacc("TRN2", target_bir_lowering=False, debug=False,
                   num_devices=N_CORES)

    # Per-core inputs (same shapes on every core, different data).
    xf = nc.dram_tensor("xf", (C, S), F32, kind="ExternalInput").ap()
    wqT = nc.dram_tensor("wqT", (C, P), F32, kind="ExternalInput").ap()
    wkT = nc.dram_tensor("wkT", (C, P), F32, kind="ExternalInput").ap()
    wvT = nc.dram_tensor("wvT", (C, O), F32, kind="ExternalInput").ap()
    wpT = nc.dram_tensor("wpT", (2, P, P), F32, kind="ExternalInput").ap()
    bq = nc.dram_tensor("bq", (P, 1), F32, kind="ExternalInput").ap()
    bv = nc.dram_tensor("bv", (1, P), F32, kind="ExternalInput").ap()
    tri = nc.dram_tensor("tri", (P, P), F32, kind="ExternalInput").ap()
    out = nc.dram_tensor("out", (O, S), F32, kind="ExternalOutput").ap()
    dbg_out = None
    if dbg:
        dbg_out = {
            nm: nc.dram_tensor(nm, shp, F32, kind="ExternalOutput").ap()
            for nm, shp in [("d_qT", (P, S)), ("d_kT", (P, S)),
                            ("d_vst", (P, 32 * HG * 33)),
                            ("d_outn", (P, S))]}

    with tile.TileContext(nc) as tc:
        with ExitStack() as ctx:
            _emit(ctx, tc, nc, xf, wqT, wkT, wvT, wpT, bq, bv, tri, out,
                  reps=reps, dbg_out=dbg_out, ablate=ablate)

    nc.compile()
    return nc


def _emit(ctx, tc, nc, xf, wqT, wkT, wvT, wpT, bq, bv, tri, out, reps=1,
          dbg_out=None, ablate=()):
    def dump(name, ap):
        if dbg_out is None or name not in dbg_out:
            return
        st = tmp.tile([P, 4224], F32, tag="big")
        fs = 1
        for _, cnt in ap.ap[1:]:
            fs *= cnt
        nc.vector.tensor_copy(st[:, 0:fs], ap)
        nc.sync.dma_start(out=dbg_out.pop(name), in_=st[:, 0:fs])

    scale = 1.0 / math.sqrt(HD)
    Exp = mybir.ActivationFunctionType.Exp

    consts = ctx.enter_context(tc.tile_pool(name="consts", bufs=1))
    qk_ps = ctx.enter_context(tc.tile_pool(name="qk_ps", bufs=2,
                                           space="PSUM"))
    av_ps = ctx.enter_context(tc.tile_pool(name="av_ps", bufs=4, space="PSUM"))
    es_pool = ctx.enter_context(tc.tile_pool(name="es", bufs=4))
    norm = ctx.enter_context(tc.tile_pool(name="norm", bufs=2))
    tmp = ctx.enter_context(tc.tile_pool(name="tmp", bufs=1))

    # ---- load constants / weights -------------------------------------
    # DMA-loaded data cannot feed FP32r matmuls directly; a compute-engine
    # copy with float32r output performs the required rounding.
    wq_ld = tmp.tile([P, 2, P], F32, tag="w")
    nc.sync.dma_start(out=wq_ld, in_=wqT.rearrange("(c p) m -> p c m", p=P))
    wq_sb = consts.tile([P, 2, P], F32R)
    nc.vector.tensor_copy(wq_sb, wq_ld)
    wk_ld = tmp.tile([P, 2, P], F32, tag="w")
    nc.sync.dma_start(out=wk_ld, in_=wkT.rearrange("(c p) m -> p c m", p=P))
    wk_sb = consts.tile([P, 2, P], F32R)
    nc.vector.tensor_copy(wk_sb, wk_ld)
    wv_ld = tmp.tile([P, 2, O], F32, tag="w")
    nc.sync.dma_start(out=wv_ld, in_=wvT.rearrange("(c p) m -> p c m", p=P))
    wv_sb = consts.tile([P, 2, O], F32R)
    nc.vector.tensor_copy(wv_sb, wv_ld)
    wp_ld = tmp.tile([P, 2, P], F32, tag="w")
    nc.sync.dma_start(out=wp_ld, in_=wpT.rearrange("m p n -> p m n"))
    wp_sb = consts.tile([P, 2, P], F32R)
    nc.vector.tensor_copy(wp_sb, wp_ld)
    # sliced load+round so the first projection matmuls start after one
    # 512-col slice instead of the whole 4 MB x transfer
    x_ld = tmp.tile([P, 2, S], F32, tag="big")  # xf as two 128-row chunks
    x_sb = consts.tile([P, 2, S], F32R)
    xr = xf.rearrange("(c p) s -> p c s", p=P)
    for sl in range(NQ):
        nc.sync.dma_start(out=x_ld[:, :, bass.ts(sl, QC)],
                          in_=xr[:, :, bass.ts(sl, QC)])
        nc.vector.tensor_copy(x_sb[:, :, bass.ts(sl, QC)],
                              x_ld[:, :, bass.ts(sl, QC)])
    bq_sb = consts.tile([P, 1], F32)
    nc.sync.dma_start(out=bq_sb, in_=bq)
    bv_row = consts.tile([1, P], F32)
    nc.sync.dma_start(out=bv_row, in_=bv)
    tri_ld = tmp.tile([P, P], F32, tag="w")
    nc.sync.dma_start(out=tri_ld, in_=tri)
    tri_sb = consts.tile([P, P], BF16)
    nc.vector.tensor_copy(tri_sb, tri_ld)

    bv_bc = consts.tile([P, P], F32)            # bv broadcast down partitions
    nc.gpsimd.partition_broadcast(bv_bc, bv_row)

    # On-device repeat loop for timing runs (reps>1): the whole compute
    # phase re-executes; consts/DMA loads stay outside.
    if reps > 1:
        loop_cm = tc.For_i(0, reps, 1)
        loop_cm.__enter__()

    # ---- persistent SBUF tiles ------------------------------------------
    qT = consts.tile([P, S], BF16)               # 4 heads x 32 dims on parts
    kT = consts.tile([P, S], BF16)
    v_st = consts.tile([P, S // KT, HG, 33], BF16)   # kpos-major v (+ ones)
    outn = consts.tile([P, S], F32R)             # normalized out^T

    nc.vector.memset(v_st[:, :, :, 32:33], 1.0)

    # mask broadcast view over the 2-head slot dim
    tri_bc = bass.AP(tensor=tri_sb.tensor, offset=tri_sb.offset,
                     ap=[tri_sb.ap[0], [0, 2], tri_sb.ap[1]])

    def emit_proj_row(j):
        """q/k projections for q-chunk j and v projection for its 4 k-tiles."""
        pt = qk_ps.tile([P, 2, QC], F32, tag="qk", name="pj_qk")
        for i, w_sb in enumerate((wq_sb, wk_sb)):
            for cc in range(2):
                nc.tensor.matmul(pt[:, i, :], w_sb[:, cc, :],
                                 x_sb[:, cc, bass.ts(j, QC)],
                                 start=(cc == 0), stop=(cc == 1))
        nc.vector.tensor_scalar_add(qT[:, bass.ts(j, QC)], pt[:, 0, :],
                                    bq_sb)
        nc.vector.tensor_copy(kT[:, bass.ts(j, QC)], pt[:, 1, :])
        vt = qk_ps.tile([P, 2, QC], F32, tag="qk", name="pj_v")
        for sub in range(4):
            st = 4 * j + sub
            ps = vt[:, sub // 2, (sub % 2) * O:(sub % 2 + 1) * O]
            for cc in range(2):
                nc.tensor.matmul(ps, x_sb[:, cc, bass.ts(st, KT)],
                                 wv_sb[:, cc, :], start=(cc == 0),
                                 stop=(cc == 1))
            nc.vector.tensor_add(
                v_st[:, st, :, 0:32],
                ps[:, 0:P].rearrange("p (h d) -> p h d", h=HG),
                bv_bc.rearrange("p (h d) -> p h d", h=HG))

    AV_LAG = 2     # AV trails QK/exp by this many k-tiles (row-start slack)

    def emit_attn_row(j, av_tiles):
        """Generator: yields once after the first k-tile's QK/exp slots."""
        nkt = 4 * j + 4
        esq = []                                  # pending (kt, es-pair list)

        def emit_av(kt, ess):
            c0 = max(0, KT * kt - QC * j)
            for pair in range(2):
                for sub in range(2):
                    h = 2 * pair + sub
                    nc.tensor.matmul(av_tiles[h][:, c0:QC],
                                     v_st[:, kt, h, 0:33],
                                     ess[pair][:, sub, c0:QC],
                                     start=(kt == 0), stop=(kt == nkt - 1))

        for kt in range(nkt):
            c0 = max(0, KT * kt - QC * j)        # causal start col (0/128/..)
            ess = []
            for pair in range(2):
                # 2 concurrent QK matmuls (PE row groups) per 2-bank slot
                pt = qk_ps.tile([P, 2, QC], F32, tag="qk", name="qk")
                for sub in range(2):
                    h = 2 * pair + sub
                    tp = (96, 0) if h == 3 else None
                    nc.tensor.matmul(
                        pt[:, sub, c0:QC],
                        kT[h * HD:(h + 1) * HD, bass.ts(kt, KT)],
                        qT[h * HD:(h + 1) * HD, j * QC + c0:(j + 1) * QC],
                        start=True, stop=True, tile_position=tp)
                es = es_pool.tile([P, 2, QC], BF16, tag="es", bufs=24)
                nc.scalar.activation(es[:, :, c0:QC], pt[:, :, c0:QC], Exp,
                                     scale=scale)
                if kt >= 4 * j and "mask" not in ablate:  # diagonal
                    sl = es[:, :, c0:c0 + KT]
                    nc.gpsimd.tensor_mul(sl, sl, tri_bc)
                ess.append(es)
            esq.append((kt, ess))
            if "av" in ablate:
                esq.pop(0)
            elif len(esq) > AV_LAG:
                emit_av(*esq.pop(0))
            if kt == 0:
                yield
        for item in esq:
            emit_av(*item)

    def emit_norm_row(j, av_tiles):
        # Normalize straight out of PSUM: denominator row -> partition 0,
        # reciprocal, gpsimd broadcast, then one DVE multiply per head
        # (av rows and rbc both partition-0-based, so bases align).
        for h in range(HG):
            av = av_tiles[h]
            l0 = norm.tile([1, QC], F32, tag="l0", name="l0", bufs=4)
            nc.vector.tensor_copy(l0, av[32:33, :])
            recip = norm.tile([1, QC], F32, tag="recip", name="recip")
            rscr = norm.tile([1, QC], F32, tag="rscr", name="rscr", bufs=1)
            nc.vector.reciprocal_approx_accurate(recip, l0, rscr)
            rbc = norm.tile([32, QC], F32, tag="rbc", name="rbc")
            nc.gpsimd.partition_broadcast(rbc, recip)
            nc.vector.tensor_mul(
                outn[h * HD:(h + 1) * HD, bass.ts(j, QC)],
                av[0:32, :], rbc)

    # ---- pipeline: projections one row ahead, norms one row behind ------
    # Per row: evacuate row j-1's AV psum early (frees the 2 av banks),
    # start row j's QK/exp stream, then slip in row j+1's projections and
    # row j-1's normalization math behind the first exp slots.
    emit_proj_row(0)
    prev = None
    for j in range(NQ):
        av_tiles = [av_ps.tile([33, QC], F32, tag="av", name=f"av{h}")
                    for h in range(HG)]
        row = emit_attn_row(j, av_tiles)
        next(row)                                 # first k-tile's QK/exp out
        if j + 1 < NQ and "proj" not in ablate:
            emit_proj_row(j + 1)
        if prev is not None and "norm" not in ablate:
            emit_norm_row(j - 1, prev)
        for _ in row:
            pass
        prev = av_tiles
    if "norm" not in ablate:
        emit_norm_row(NQ - 1, prev)
    dump("d_qT", qT[:, :])
    dump("d_kT", kT[:, :])
    dump("d_vst", v_st.rearrange("p a b c -> p (a b c)"))
    dump("d_outn", outn[:, :])

    # ---- output projection: out = Wp[:, our 128 cols] @ outn ------------
    for j in (() if "oproj" in ablate else range(NQ)):
        ot = qk_ps.tile([P, 2, QC], F32, tag="qk", name="op")
        for m in range(2):
            nc.tensor.matmul(ot[:, m, :], wp_sb[:, m, :],
                             outn[:, bass.ts(j, QC)], start=True, stop=True)
            ob = norm.tile([P, QC], F32, tag="ob", name="ob", bufs=4)
            nc.vector.tensor_copy(ob, ot[:, m, :])
            nc.sync.dma_start(
                out=out.rearrange("(m p) s -> p m s", p=P)[:, m,
                                                           bass.ts(j, QC)],
                in_=ob)

    if reps > 1:
        loop_cm.__exit__(None, None, None)


_BUILT = {}


def _get_built(reps=1):
    if reps not in _BUILT:
        _BUILT[reps] = build_kernel(reps)
    return _BUILT[reps]


def make_in_maps(x, Wq, bq, Wkv, bkv, Wp, bp):
    x = np.asarray(x, dtype=np.float32)
    Wq = np.asarray(Wq, dtype=np.float32)
    bq = np.asarray(bq, dtype=np.float32)
    Wkv = np.asarray(Wkv, dtype=np.float32)
    Wp = np.asarray(Wp, dtype=np.float32)

    Wk, Wv = Wkv[:E], Wkv[E:]
    bv_ = np.asarray(bkv, dtype=np.float32)[E:]

    # causal triangle in transposed-score orientation (kpos partition,
    # qpos free): valid when qpos >= kpos
    tri_np = (np.arange(P)[None, :] >= np.arange(P)[:, None]).astype(
        np.float32)

    in_maps = []
    for c in range(N_CORES):
        n, hg = c // 2, c % 2
        rows = slice(hg * P, (hg + 1) * P)
        # rotate wvT columns so this core's 128 head columns sit at 0:128
        wvT_c = np.ascontiguousarray(np.roll(Wv.T, -hg * P, axis=1))
        in_maps.append({
            "xf": np.ascontiguousarray(x[n].reshape(C, S)),
            "wqT": np.ascontiguousarray(Wq[rows].T),
            "wkT": np.ascontiguousarray(Wk[rows].T),
            "wvT": wvT_c,
            "wpT": np.ascontiguousarray(
                Wp[:, rows].reshape(2, P, P).transpose(0, 2, 1)),
            "bq": np.ascontiguousarray(bq[rows, None]),
            "bv": np.ascontiguousarray(bv_[None, rows]),
            "tri": tri_np,
        })
    return in_maps


def kernel(x, Wq, bq, Wkv, bkv, Wp, bp, n_heads):
    assert int(n_heads) == H
    bp = np.asarray(bp, dtype=np.float32)

    from concourse.bass_utils import run_bass_kernel_spmd

    nc = _get_built()
    in_maps = make_in_maps(x, Wq, bq, Wkv, bkv, Wp, bp)

    res = run_bass_kernel_spmd(nc, in_maps, core_ids=list(range(N_CORES)))

    outp = np.zeros((N, O, S), np.float32)
    for c in range(N_CORES):
        outp[c // 2] += res.results[c]["out"]
    outp += bp[None, :, None]
    return outp.reshape(N, O, HH, WW)
